# revision 32
# baseline (speedup 1.0000x reference)
"""GPT-2 block (B=2, T=2048, C=768, H=12) on 8 Trainium2 NeuronCores.

Sharding: data-parallel over batch (2) x 4-way query-tile split per batch.
Each core computes K/V for its full batch (avoids on-chip collectives,
whose latency floor exceeds the redundant compute) and runs attention +
MLP for 4 of the 16 query tiles, interleaved {g, 7-g, 8+g, 15-g} so the
causal-attention work is identical across cores.

The SPMD program is uniform across cores: per-core differences are pushed
into the data via a k-tile permutation of the sequence (each core's query
tiles sit at fixed positions {3,7,11,15}; every tile's causal prefix is
placed before it) plus per-core causal masks.

Layouts: activations enter matmuls feature-major (xnT [C,T]) so QKV needs
no transposes; attention scores are computed transposed (S^T [k,q]) so
exp(S^T) is directly the stationary operand of the A*V matmul, and a ones
column appended to V produces the softmax denominator in the same matmul.

Precision: weights and matmul activations are fp8 e4m3 with DoubleRow
matmuls (2 K-rows/cycle, K-tiles of 256). Weights are pre-scaled by 32 on
the host to stay in the fp8 normal range; the scale folds into the exp()
argument for attention (q,k both 32x -> scale/1024), into the V ones
column (=32 so softmax numerator/denominator cancel), and into one cheap
descale per MLP/proj output. Attention S/AV matmuls stay bf16.
"""

import sys

sys.path.insert(0, "/opt/trn_rl_repo")

import numpy as np
import ml_dtypes

import bass_rust
import concourse.bass as bass
import concourse.bacc as bacc
import concourse.tile as tile
from concourse import mybir
from concourse.vector_clock import ScopedClock

BF16 = ml_dtypes.bfloat16
F32 = mybir.dt.float32
BF = mybir.dt.bfloat16
F8 = mybir.dt.float8e4
NP_F8 = mybir.dt.np(F8)

B, T, C, H = 2, 2048, 768, 12
HD = C // H  # 64
DFF = 4 * C  # 3072
TT = T // 128  # 16 token tiles
CT = C // 128  # 6 feature tiles
KT = C // 256  # 3 DoubleRow k-tiles over C
KT2 = DFF // 256  # 12 DoubleRow k-tiles over DFF
FT = DFF // 128  # 24
QPOS = (3, 7, 11, 15)  # fixed positions of this core's query tiles
NQ = 512  # queries per core
WS = 32.0  # fp8 weight pre-scale
AF = mybir.ActivationFunctionType
ALU = mybir.AluOpType
DR = mybir.MatmulPerfMode.DoubleRow

# ---------------------------------------------------------------------------
# Tile exit-drain fix: the final SP drain carries one wait per live logical
# processor, but TRN2 ISA instructions hold at most 1 embedded sync wait in
# this toolchain. Split the waits across a chain of SP drains.
# ---------------------------------------------------------------------------
_MAX_WAITS = 1


def _drain_and_barrier(self, tick_clock, wait_clock):
    drain_inst = self.nc.sync.drain()
    wait_clock.add_sem_waits(
        drain_inst.ins, ScopedClock({None: tick_clock.global_clock})
    )
    si = drain_inst.ins.sync_info
    if si is not None and len(si.on_wait) > _MAX_WAITS:
        waits = list(si.on_wait)
        drain_inst.ins.sync_info = bass_rust.SyncInfo(
            on_wait=waits[:_MAX_WAITS], on_update=list(si.on_update)
        )
        rest = waits[_MAX_WAITS:]
        for i in range(0, len(rest), _MAX_WAITS):
            extra = self.nc.sync.drain()
            extra.ins.sync_info = bass_rust.SyncInfo(
                on_wait=rest[i : i + _MAX_WAITS], on_update=[]
            )
    self.nc.all_engine_barrier()
    assert self.sems is not None
    popped = self.nc._tile_sem_poison_stack.pop()
    assert popped is self._sem_poison
    self.nc.clear_and_free_semaphores(list(self.sems.allocated().values()))
    self.nc.all_engine_barrier()


tile.TileContext._drain_and_barrier = _drain_and_barrier


# ---------------------------------------------------------------------------
# Per-core sharding layout (host side)
# ---------------------------------------------------------------------------
def core_layout(g):
    """For group index g (0..3): (qtiles sorted, perm) with the core's query
    tiles at positions QPOS and every tile's causal prefix placed before it."""
    qtiles = sorted([g, 7 - g, 8 + g, 15 - g])
    posmap = dict(zip(QPOS, qtiles))
    rest = iter([t for t in range(TT) if t not in qtiles])
    perm = [posmap[p] if p in posmap else next(rest) for p in range(TT)]
    # causal validity: tiles <= qtiles[j] all sit at positions <= QPOS[j]
    for j, a in enumerate(qtiles):
        assert set(range(a + 1)) <= set(perm[: QPOS[j] + 1]), (g, j, perm)
    return qtiles, perm


def core_masks(qtiles, perm):
    """masks[kp] = causal mask of k-position kp against query tile j=kp//4
    (the first in-suffix block - across all core layouts the only block
    that is ever not all-ones)."""
    masks = np.zeros((TT, 128, 128), dtype=BF16)
    for kp in range(TT):
        tk = perm[kp] * 128 + np.arange(128)[:, None]
        a = qtiles[kp // 4]
        tq = a * 128 + np.arange(128)[None, :]
        masks[kp] = (tk <= tq).astype(BF16)
    return masks


def pack_dr(W):
    """[K, N] fp32 -> DoubleRow-paired fp8 [K/256, 128, 2, N], pre-scaled.
    Logical k = 256*kt + 128*r + p."""
    K, N = W.shape
    Wp = (np.asarray(W, np.float32) * WS).reshape(K // 256, 2, 128, N)
    return np.ascontiguousarray(Wp.transpose(0, 2, 1, 3)).astype(NP_F8)


# ---------------------------------------------------------------------------
# The Bass program (identical for all 8 cores)
# ---------------------------------------------------------------------------
def build_program():
    nc = bacc.Bacc("TRN2")

    d_x = nc.dram_tensor("x_perm", [T, C], F32, kind="ExternalInput")
    d_xob = nc.dram_tensor("x_own_b", [NQ, C], F32, kind="ExternalInput")
    d_masks = nc.dram_tensor("masks", [TT, 128, 128], BF, kind="ExternalInput")
    d_wq = nc.dram_tensor("wq", [KT, 128, 2, C], F8, kind="ExternalInput")
    d_wk = nc.dram_tensor("wk", [KT, 128, 2, C], F8, kind="ExternalInput")
    d_wv = nc.dram_tensor("wv", [KT, 128, 2, C], F8, kind="ExternalInput")
    d_wp = nc.dram_tensor("wp", [C, C], BF, kind="ExternalInput")
    d_wfc = nc.dram_tensor("wfc", [C, DFF], BF, kind="ExternalInput")
    d_wfc2 = nc.dram_tensor("wfc2", [DFF, C], BF, kind="ExternalInput")
    # small per-partition constants packed into one tensor:
    # [l1g l1b l2g l2b bq bk] each [128, CT]
    d_cn = nc.dram_tensor("consts6", [128, 6 * CT], F32, kind="ExternalInput")
    # [bv_bc bfc2_bc] packed: [128, 2*C]
    d_cb = nc.dram_tensor("constsb", [128, 2 * C], F32, kind="ExternalInput")
    d_bfc = nc.dram_tensor("bfc2d", [128, FT], F32, kind="ExternalInput")
    d_ident = nc.dram_tensor("ident", [128, 128], BF, kind="ExternalInput")
    d_out = nc.dram_tensor("out", [NQ, C], F32, kind="ExternalOutput")

    with tile.TileContext(nc) as tc:
        _body(nc, tc, locals())
    nc.compile()
    return nc


def _ln_tile(nc, pool, x_ap, eps):
    """LayerNorm stats for one [128, C] fp32 tile -> (mu, rstd) [128,1]."""
    stats = pool.tile([128, 3, 6], F32, tag="bnstats", name="bnstats")
    xg = x_ap.rearrange("p (a b) -> p a b", b=256)
    for a in range(3):
        nc.vector.bn_stats(out=stats[:, a, :], in_=xg[:, a, :])
    mv = pool.tile([128, 2], F32, tag="bnaggr", name="bnaggr")
    nc.vector.bn_aggr(out=mv[:], in_=stats[:])
    sd = pool.tile([128, 1], F32, tag="sd", name="sd")
    nc.scalar.activation(out=sd[:], in_=mv[:, 1:2], func=AF.Sqrt, bias=eps[:])
    rstd = pool.tile([128, 1], F32, tag="rstd", name="rstd")
    nc.vector.reciprocal(out=rstd[:], in_=sd[:])
    return mv[:, 0:1], rstd


def _body(nc, tc, d):
    def pool(name, **kw):
        return tc.tile_pool(name=name, **kw)

    with (
        pool("const", bufs=1) as constp,
        pool("persist", bufs=1) as pers,
        pool("small", bufs=6) as small,
    ):
        # ---- constants (batched DMAs) -----------------------------------
        ident = constp.tile([128, 128], BF)
        nc.sync.dma_start(ident[:], d["d_ident"][:])
        eps = constp.tile([128, 1], F32)
        nc.vector.memset(eps[:], 1e-5)
        cn = constp.tile([128, 6, CT], F32)
        nc.sync.dma_start(cn[:], d["d_cn"][:].rearrange("p (a b) -> p a b", b=CT))
        l1g, l1b, l2g, l2b, bq, bk = (cn[:, i, :] for i in range(6))
        cb = constp.tile([128, 2, C], F32)
        nc.sync.dma_start(cb[:], d["d_cb"][:].rearrange("p (a b) -> p a b", b=C))
        bv_bc, bfc2_bc = cb[:, 0, :], cb[:, 1, :]

        # ---- persistent activations -----------------------------------
        wp_t = pers.tile([128, CT, C], BF, tag="wp", name="wp_t")
        wp = [wp_t[:, t, :] for t in range(CT)]
        wfc_t = pers.tile([128, CT, DFF], BF, tag="wfc", name="wfc_t")
        wfc = [wfc_t[:, t, :] for t in range(CT)]
        xo_t = pers.tile([128, 4, C], F32, tag="xo", name="xo_t")
        x_own = [xo_t[:, j, :] for j in range(4)]
        qT = [pers.tile([128, NQ], BF, tag=f"qT{t}", name=f"qT{t}") for t in range(CT)]
        yT = [pers.tile([128, NQ], BF, tag=f"yT{t}", name=f"yT{t}") for t in range(CT)]

        with pool("attn_sb", bufs=1) as attnp:
            kT = [attnp.tile([128, T], BF, tag=f"kT{t}", name=f"kT{t}") for t in range(CT)]
            V = [attnp.tile([128, H * (HD + 32), ], BF, tag=f"V{t}", name=f"V{t}") for t in range(TT)]
            masks_t = attnp.tile([128, TT, 128], BF, tag="masks", name="masks_t")
            masks = [masks_t[:, t, :] for t in range(TT)]

            # ======== phase 1: LN1 + transpose to feature-major fp8 ========
            with (
                pool("ph1", bufs=1) as ph1p,
                pool("ph1s", bufs=4) as ph1s,
                pool("wqkv", bufs=1) as wp_,
            ):
                xnT8 = [ph1p.tile([128, 2, T], F8, tag=f"xnT{t}", name=f"xnT{t}") for t in range(KT)]
                xnTq8 = [ph1p.tile([128, 2, NQ], F8, tag=f"xnTq{t}", name=f"xnTq{t}") for t in range(KT)]
                wq8 = [wp_.tile([128, 2, C], F8, tag=f"wq{t}", name=f"wq{t}") for t in range(KT)]
                wk8 = [wp_.tile([128, 2, C], F8, tag=f"wk{t}", name=f"wk{t}") for t in range(KT)]
                wv8 = [wp_.tile([128, 2, C], F8, tag=f"wv{t}", name=f"wv{t}") for t in range(KT)]
                for t in range(KT):
                    nc.sync.dma_start(wk8[t][:], d["d_wk"][t, :, :, :])
                for t in range(KT):
                    nc.sync.dma_start(wv8[t][:], d["d_wv"][t, :, :, :])
                for t in range(KT):
                    nc.sync.dma_start(wq8[t][:], d["d_wq"][t, :, :, :])
                nc.sync.dma_start(
                    masks_t[:], d["d_masks"][:].rearrange("k p c -> p k c")
                )
                nc.sync.dma_start(
                    wp_t[:], d["d_wp"][:].rearrange("(t p) c -> p t c", p=128)
                )
                nc.sync.dma_start(
                    wfc_t[:], d["d_wfc"][:].rearrange("(t p) c -> p t c", p=128)
                )
                nc.gpsimd.dma_start(
                    xo_t[:], d["d_xob"][:].rearrange("(t p) c -> p t c", p=128)
                )
                with (
                    pool("ph1t", bufs=1, space="PSUM") as ph1t,
                    pool("ph2k", bufs=2, space="PSUM") as ph2k,
                ):
                    # PE warm-up: keep the HAM activity monitor at full clock
                    # while the first LayerNorm group is still in flight
                    warm = ph2k.tile([128, 512], F32, tag="pqk", name="warm")
                    for _ in range(128):
                        nc.tensor.matmul(warm[:, 0:128], ident[:], ident[:])
                    for ttg in range(4):
                        ptb = [
                            ph1t.tile([128, 512], BF, tag=f"ptb{t}", name=f"ptb{t}")
                            for t in range(CT)
                        ]
                        for ti in range(4):
                            tt = ttg * 4 + ti
                            xt_t = ph1s.tile([128, C], F32, tag="xt", name="xt")
                            nc.scalar.dma_start(
                                xt_t[:], d["d_x"][tt * 128 : (tt + 1) * 128, :]
                            )
                            xt = xt_t[:]
                            mu, rstd = _ln_tile(nc, small, xt, eps)
                            xn = ph1s.tile([128, C], BF, tag="xn", name="xn")
                            nc.vector.tensor_scalar(
                                out=xn[:], in0=xt, scalar1=mu, scalar2=rstd[:],
                                op0=ALU.subtract, op1=ALU.mult,
                            )
                            for ct in range(CT):
                                nc.tensor.transpose(
                                    ptb[ct][:, ti * 128 : (ti + 1) * 128],
                                    xn[:, ct * 128 : (ct + 1) * 128], ident[:],
                                )
                        for ct in range(CT):
                            kt, r = ct // 2, ct % 2
                            nc.scalar.activation(
                                out=xnT8[kt][:, r, ttg * 512 : (ttg + 1) * 512],
                                in_=ptb[ct][:], func=AF.Identity,
                                scale=l1g[:, ct : ct + 1], bias=l1b[:, ct : ct + 1],
                            )
                            # own q-tile of this group sits at ti == 3
                            nc.scalar.activation(
                                out=xnTq8[kt][:, r, ttg * 128 : (ttg + 1) * 128],
                                in_=ptb[ct][:, 384:512], func=AF.Identity,
                                scale=l1g[:, ct : ct + 1], bias=l1b[:, ct : ct + 1],
                            )
                        # kT chunk ttg depends only on this token-group
                        for f in range(CT):
                            ps = ph2k.tile([128, 512], F32, tag="pqk", name="pk")
                            for kt in range(KT):
                                nc.tensor.matmul(
                                    ps[:], wk8[kt][:, :, f * 128 : (f + 1) * 128],
                                    xnT8[kt][:, :, ttg * 512 : (ttg + 1) * 512],
                                    start=(kt == 0), stop=(kt == KT - 1),
                                    perf_mode=DR,
                                )
                            nc.vector.tensor_scalar(
                                out=kT[f][:, ttg * 512 : (ttg + 1) * 512],
                                in0=ps[:], scalar1=bk[:, f : f + 1],
                                scalar2=None, op0=ALU.add,
                            )

                # ======== phase 2: Q^T, V (fp8 DoubleRow) ========
                with (
                    pool("ph2ps", bufs=3, space="PSUM") as ph2ps,
                    pool("ph2pv", bufs=2, space="PSUM") as ph2pv,
                ):
                    for nn in range(4):
                        for tt in range(nn * 4, nn * 4 + 4):
                            pv = ph2pv.tile([128, C], F32, tag="pv", name="pv")
                            for lo, hi in ((0, 512), (512, 768)):
                                for kt in range(KT):
                                    nc.tensor.matmul(
                                        pv[:, lo:hi],
                                        xnT8[kt][:, :, tt * 128 : (tt + 1) * 128],
                                        wv8[kt][:, :, lo:hi],
                                        start=(kt == 0), stop=(kt == KT - 1),
                                        perf_mode=DR,
                                    )
                            vt = V[tt][:].rearrange("p (h e) -> p h e", e=HD + 32)
                            # 32 replicated "ones" columns per head: the AV
                            # matmul then lands the softmax denominator in
                            # psum rows 64:96 (value WS cancels the fp8
                            # weight scale between numerator and denominator)
                            nc.vector.memset(vt[:, :, HD : HD + 32], WS)
                            pvh = pv[:].rearrange("p (h e) -> p h e", e=HD)
                            nc.vector.tensor_tensor(
                                out=vt[:, :, 0:HD], in0=pvh[:],
                                in1=bv_bc[:].rearrange("p (h e) -> p h e", e=HD),
                                op=ALU.add,
                            )
                    # qT[f] [128, 512] = (Wq[:, f].T @ xnTq) + bq
                    for f in range(CT):
                        ps = ph2ps.tile([128, NQ], F32, tag="pqk", name="pq")
                        for kt in range(KT):
                            nc.tensor.matmul(
                                ps[:], wq8[kt][:, :, f * 128 : (f + 1) * 128],
                                xnTq8[kt][:], start=(kt == 0), stop=(kt == KT - 1),
                                perf_mode=DR,
                            )
                        nc.vector.tensor_scalar(
                            out=qT[f][:], in0=ps[:], scalar1=bq[:, f : f + 1],
                            scalar2=None, op0=ALU.add,
                        )

            # ======== phase 3: attention (bf16, 4 heads in flight) ========
            # exp is batched over head PAIRS (one ACT op per pair) since ACT
            # runs 1x with a 352-cycle fixed cost per instruction.
            with (
                pool("ph3", bufs=8) as ph3s,
                pool("ph3ps", bufs=2, space="PSUM") as ph3ps,
                pool("ph3pa", bufs=1, space="PSUM") as ph3pa,
            ):
                for hg in range(H // 4):
                    hs = [hg * 4 + i for i in range(4)]
                    pavs = {
                        h: ph3pa.tile(
                            [128, NQ], F32, tag=f"pav{h % 4}", name=f"pav{h % 4}"
                        )
                        for h in hs
                    }
                    for kp in range(TT):
                        cs = 128 * (kp // 4)
                        psbs = {}
                        for pi in range(2):
                            hA, hB = hs[2 * pi], hs[2 * pi + 1]
                            ps2 = ph3ps.tile([128, 2, NQ], F32, tag="ps2", name="ps2")
                            for r, h in ((0, hA), (1, hB)):
                                ro = (h % 2) * 64
                                nc.tensor.matmul(
                                    ps2[:, r, cs:NQ],
                                    kT[h // 2][ro : ro + 64, kp * 128 : (kp + 1) * 128],
                                    qT[h // 2][ro : ro + 64, cs:NQ],
                                )
                            p_sb = ph3s.tile([128, 2, NQ], BF, tag="p_sb", name="p_sb")
                            # q,k both carry the 32x fp8 weight scale
                            nc.scalar.activation(
                                out=p_sb[:, :, cs:NQ], in_=ps2[:, :, cs:NQ],
                                func=AF.Exp, scale=0.125 / (WS * WS),
                            )
                            # only the first in-suffix 128-col block is ever
                            # not all-ones (across every core layout)
                            for r, h in ((0, hA), (1, hB)):
                                nc.vector.tensor_mul(
                                    p_sb[:, r, cs : cs + 128],
                                    p_sb[:, r, cs : cs + 128], masks[kp][:],
                                )
                                psbs[h] = (p_sb, r)
                        for h in hs:
                            p_sb, r = psbs[h]
                            nc.tensor.matmul(
                                pavs[h][0 : HD + 32, cs:NQ],
                                V[kp][:, h * (HD + 32) : (h + 1) * (HD + 32)],
                                p_sb[:, r, cs:NQ],
                                start=(kp == 0), stop=(kp == TT - 1),
                                skip_group_check=True,
                            )
                    # gather the 4 heads' denominators and invert them in a
                    # single wide reciprocal (DVE reciprocal cost is per
                    # free-element, independent of partition count)
                    denall = ph3s.tile([128, NQ], F32, tag="denall", name="denall")
                    for i, h in enumerate(hs):
                        nc.vector.tensor_copy(
                            denall[32 * i : 32 * (i + 1), :],
                            pavs[h][HD : HD + 32, :],
                        )
                    rball = ph3s.tile([128, NQ], F32, tag="rball", name="rball")
                    nc.vector.reciprocal(out=rball[:], in_=denall[:])
                    for i, h in enumerate(hs):
                        ro = (h % 2) * 64
                        for half in range(2):
                            nc.vector.tensor_tensor(
                                out=yT[h // 2][ro + 32 * half : ro + 32 * (half + 1), :],
                                in0=pavs[h][32 * half : 32 * (half + 1), :],
                                in1=rball[32 * i : 32 * (i + 1), :], op=ALU.mult,
                            )

                # keep the PE activity monitor at full clock through the
                # softmax tail so the MLP phases start warm
                warm2 = ph3ps.tile([128, 2, NQ], F32, tag="ps2", name="warm2")
                for _ in range(96):
                    nc.tensor.matmul(warm2[:, 0, 0:128], ident[:], ident[:])

        # ======== phase 4: proj + residual + LN2 ========
        with pool("mlp_sb", bufs=1) as mlpp:
            wfc2A = mlpp.tile([128, FT // 2, C], BF, tag="wfc2A", name="wfc2A")
            nc.sync.dma_start(
                wfc2A[:],
                d["d_wfc2"][: FT // 2 * 128, :].rearrange("(t p) c -> p t c", p=128),
            )
            x2 = [mlpp.tile([128, C], F32, tag=f"x2{j}", name=f"x2{j}") for j in range(4)]
            xn2T = [mlpp.tile([128, NQ], BF, tag=f"xn2T{t}", name=f"xn2T{t}") for t in range(CT)]
            hT = [mlpp.tile([128, NQ], BF, tag=f"hT{t}", name=f"hT{t}") for t in range(FT)]
            with (
                pool("mlp1", bufs=1) as m1p,
                pool("mlp1s", bufs=3) as m1s,
            ):
                bfcc = m1p.tile([128, FT], F32, tag="bfcc", name="bfcc")
                nc.sync.dma_start(bfcc[:], d["d_bfc"][:])

                with (
                    pool("ph4p", bufs=2, space="PSUM") as ph4p,
                    pool("ph4t", bufs=4, space="PSUM") as ph4t,
                ):
                    for qt in range(4):
                        pp = ph4p.tile([128, C], F32, tag="pp", name="pp")
                        for lo, hi in ((0, 512), (512, 768)):
                            for ct in range(CT):
                                nc.tensor.matmul(
                                    pp[:, lo:hi],
                                    yT[ct][:, qt * 128 : (qt + 1) * 128],
                                    wp[ct][:, lo:hi],
                                    start=(ct == 0), stop=(ct == CT - 1),
                                )
                        nc.vector.tensor_add(x2[qt][:], pp[:], x_own[qt][:])
                        mu, rstd = _ln_tile(nc, small, x2[qt][:], eps)
                        xn2 = m1s.tile([128, C], BF, tag="xn2", name="xn2")
                        nc.vector.tensor_scalar(
                            out=xn2[:], in0=x2[qt][:], scalar1=mu, scalar2=rstd[:],
                            op0=ALU.subtract, op1=ALU.mult,
                        )
                        for ct in range(CT):
                            pt = ph4t.tile([128, 128], BF, tag="pt4", name="pt4")
                            nc.tensor.transpose(
                                pt[:], xn2[:, ct * 128 : (ct + 1) * 128], ident[:]
                            )
                            nc.scalar.activation(
                                out=xn2T[ct][:, qt * 128 : (qt + 1) * 128],
                                in_=pt[:], func=AF.Identity,
                                scale=l2g[:, ct : ct + 1], bias=l2b[:, ct : ct + 1],
                            )

                # ======== phase 5: fc -> hT directly (feature-major out),
                # gelu bias is then per-partition ========
                with pool("ph5p", bufs=3, space="PSUM") as ph5p:
                    for f in range(FT):
                        ph_ = ph5p.tile([128, 512], F32, tag="ph5", name="ph5")
                        for ct in range(CT):
                            nc.tensor.matmul(
                                ph_[:],
                                wfc[ct][:, f * 128 : (f + 1) * 128],
                                xn2T[ct][:],
                                start=(ct == 0), stop=(ct == CT - 1),
                            )
                        nc.scalar.activation(
                            out=hT[f][:], in_=ph_[:], func=AF.Gelu_apprx_tanh,
                            bias=bfcc[:, f : f + 1],
                        )

            # ======== phase 7: fc2 + residual + out ========
            with (
                pool("mlp2", bufs=1) as m2p,
                pool("mlp2s", bufs=3) as m2s,
                pool("ph7p", bufs=2, space="PSUM") as ph7p,
            ):
                wfc2B = m2p.tile([128, FT // 2, C], BF, tag="wfc2B", name="wfc2B")
                nc.sync.dma_start(
                    wfc2B[:],
                    d["d_wfc2"][FT // 2 * 128 :, :].rearrange(
                        "(t p) c -> p t c", p=128
                    ),
                )
                wfc2 = [wfc2A[:, t, :] for t in range(FT // 2)] + [
                    wfc2B[:, t, :] for t in range(FT // 2)
                ]
                for qt in range(4):
                    po = ph7p.tile([128, C], F32, tag="po", name="po")
                    for lo, hi in ((0, 512), (512, 768)):
                        for kt in range(FT):
                            nc.tensor.matmul(
                                po[:, lo:hi],
                                hT[kt][:, qt * 128 : (qt + 1) * 128],
                                wfc2[kt][:, lo:hi],
                                start=(kt == 0), stop=(kt == FT - 1),
                            )
                    t1 = m2s.tile([128, C], F32, tag="t1", name="t1")
                    nc.vector.tensor_add(t1[:], po[:], bfc2_bc[:])
                    ot = m2s.tile([128, C], F32, tag="ot", name="ot")
                    nc.vector.tensor_add(ot[:], t1[:], x2[qt][:])
                    nc.sync.dma_start(
                        d["d_out"][qt * 128 : (qt + 1) * 128, :], ot[:]
                    )


# ---------------------------------------------------------------------------
# Host-side wrapper
# ---------------------------------------------------------------------------
_PROGRAM = None


def _get_program():
    global _PROGRAM
    if _PROGRAM is None:
        _PROGRAM = build_program()
    return _PROGRAM


def make_in_maps(x, ln1_g, ln1_b, W_attn, b_attn, W_proj, b_proj,
                 ln2_g, ln2_b, W_fc, b_fc, W_fc2, b_fc2):
    x = np.asarray(x, np.float32)
    shared = {
        "wq": pack_dr(W_attn[:, 0:C]),
        "wk": pack_dr(W_attn[:, C : 2 * C]),
        "wv": pack_dr(W_attn[:, 2 * C : 3 * C]),
        "wp": np.asarray(W_proj, BF16),
        "wfc": np.asarray(W_fc, BF16),
        "wfc2": np.asarray(W_fc2, BF16),
        # consts6 = [l1g l1b l2g l2b bq bk]; q/k/v biases ride the 32x scale
        "consts6": np.concatenate([
            np.asarray(v, np.float32).reshape(CT, 128).T
            for v in (ln1_g, ln1_b, ln2_g, ln2_b,
                      np.asarray(b_attn[0:C], np.float32) * WS,
                      np.asarray(b_attn[C : 2 * C], np.float32) * WS)
        ], axis=1).copy(),
        "constsb": np.concatenate([
            np.broadcast_to(
                np.asarray(b_attn[2 * C : 3 * C], np.float32) * WS, (128, C)),
            np.broadcast_to(np.asarray(b_fc2, np.float32), (128, C)),
        ], axis=1).copy(),
        "bfc2d": np.ascontiguousarray(
            np.asarray(b_fc, np.float32).reshape(FT, 128).T),
        "ident": np.eye(128, dtype=BF16),
    }
    bp = np.asarray(b_proj, np.float32)
    in_maps, layouts = [], []
    for core in range(8):
        b, g = core // 4, core % 4
        qtiles, perm = core_layout(g)
        idx = np.concatenate([np.arange(t * 128, (t + 1) * 128) for t in perm])
        own = np.concatenate([np.arange(t * 128, (t + 1) * 128) for t in qtiles])
        m = dict(shared)
        m["x_perm"] = np.ascontiguousarray(x[b][idx])
        m["x_own_b"] = np.ascontiguousarray(x[b][own] + bp)
        m["masks"] = core_masks(qtiles, perm)
        in_maps.append(m)
        layouts.append((b, own))
    return in_maps, layouts


def unshard(results, layouts):
    out = np.empty((B, T, C), np.float32)
    for r, (b, own) in zip(results, layouts):
        out[b][own] = r["out"]
    return out


def kernel(**inputs):
    from concourse.bass_utils import run_bass_kernel_spmd

    nc = _get_program()
    in_maps, layouts = make_in_maps(**inputs)
    res = run_bass_kernel_spmd(nc, in_maps, core_ids=list(range(8)))
    return unshard(res.results, layouts)


# revision 33
# speedup vs baseline: 1.0108x; 1.0108x over previous
"""GPT-2 block (B=2, T=2048, C=768, H=12) on 8 Trainium2 NeuronCores.

Sharding: data-parallel over batch (2) x 4-way query-tile split per batch.
Each core computes K/V for its full batch (avoids on-chip collectives,
whose latency floor exceeds the redundant compute) and runs attention +
MLP for 4 of the 16 query tiles, interleaved {g, 7-g, 8+g, 15-g} so the
causal-attention work is identical across cores.

The SPMD program is uniform across cores: per-core differences are pushed
into the data via a k-tile permutation of the sequence (each core's query
tiles sit at fixed positions {3,7,11,15}; every tile's causal prefix is
placed before it) plus per-core causal masks.

Layouts: activations enter matmuls feature-major (xnT [C,T]) so QKV needs
no transposes; attention scores are computed transposed (S^T [k,q]) so
exp(S^T) is directly the stationary operand of the A*V matmul, and a ones
column appended to V produces the softmax denominator in the same matmul.

Precision: weights and matmul activations are fp8 e4m3 with DoubleRow
matmuls (2 K-rows/cycle, K-tiles of 256). Weights are pre-scaled by 32 on
the host to stay in the fp8 normal range; the scale folds into the exp()
argument for attention (q,k both 32x -> scale/1024), into the V ones
column (=32 so softmax numerator/denominator cancel), and into one cheap
descale per MLP/proj output. Attention S/AV matmuls stay bf16.
"""

import sys

sys.path.insert(0, "/opt/trn_rl_repo")

import numpy as np
import ml_dtypes

import bass_rust
import concourse.bass as bass
import concourse.bacc as bacc
import concourse.tile as tile
from concourse import mybir
from concourse.vector_clock import ScopedClock

BF16 = ml_dtypes.bfloat16
F32 = mybir.dt.float32
BF = mybir.dt.bfloat16
F8 = mybir.dt.float8e4
NP_F8 = mybir.dt.np(F8)

B, T, C, H = 2, 2048, 768, 12
HD = C // H  # 64
DFF = 4 * C  # 3072
TT = T // 128  # 16 token tiles
CT = C // 128  # 6 feature tiles
KT = C // 256  # 3 DoubleRow k-tiles over C
KT2 = DFF // 256  # 12 DoubleRow k-tiles over DFF
FT = DFF // 128  # 24
QPOS = (3, 7, 11, 15)  # fixed positions of this core's query tiles
NQ = 512  # queries per core
WS = 32.0  # fp8 weight pre-scale
AF = mybir.ActivationFunctionType
ALU = mybir.AluOpType
DR = mybir.MatmulPerfMode.DoubleRow

# ---------------------------------------------------------------------------
# Tile exit-drain fix: the final SP drain carries one wait per live logical
# processor, but TRN2 ISA instructions hold at most 1 embedded sync wait in
# this toolchain. Split the waits across a chain of SP drains.
# ---------------------------------------------------------------------------
_MAX_WAITS = 1


def _drain_and_barrier(self, tick_clock, wait_clock):
    drain_inst = self.nc.sync.drain()
    wait_clock.add_sem_waits(
        drain_inst.ins, ScopedClock({None: tick_clock.global_clock})
    )
    si = drain_inst.ins.sync_info
    if si is not None and len(si.on_wait) > _MAX_WAITS:
        waits = list(si.on_wait)
        drain_inst.ins.sync_info = bass_rust.SyncInfo(
            on_wait=waits[:_MAX_WAITS], on_update=list(si.on_update)
        )
        rest = waits[_MAX_WAITS:]
        for i in range(0, len(rest), _MAX_WAITS):
            extra = self.nc.sync.drain()
            extra.ins.sync_info = bass_rust.SyncInfo(
                on_wait=rest[i : i + _MAX_WAITS], on_update=[]
            )
    self.nc.all_engine_barrier()
    assert self.sems is not None
    popped = self.nc._tile_sem_poison_stack.pop()
    assert popped is self._sem_poison
    self.nc.clear_and_free_semaphores(list(self.sems.allocated().values()))
    self.nc.all_engine_barrier()


tile.TileContext._drain_and_barrier = _drain_and_barrier


# ---------------------------------------------------------------------------
# Per-core sharding layout (host side)
# ---------------------------------------------------------------------------
def core_layout(g):
    """For group index g (0..3): (qtiles sorted, perm) with the core's query
    tiles at positions QPOS and every tile's causal prefix placed before it."""
    qtiles = sorted([g, 7 - g, 8 + g, 15 - g])
    posmap = dict(zip(QPOS, qtiles))
    rest = iter([t for t in range(TT) if t not in qtiles])
    perm = [posmap[p] if p in posmap else next(rest) for p in range(TT)]
    # causal validity: tiles <= qtiles[j] all sit at positions <= QPOS[j]
    for j, a in enumerate(qtiles):
        assert set(range(a + 1)) <= set(perm[: QPOS[j] + 1]), (g, j, perm)
    return qtiles, perm


def core_masks(qtiles, perm):
    """masks[kp] = causal mask of k-position kp against query tile j=kp//4
    (the first in-suffix block - across all core layouts the only block
    that is ever not all-ones)."""
    masks = np.zeros((TT, 128, 128), dtype=BF16)
    for kp in range(TT):
        tk = perm[kp] * 128 + np.arange(128)[:, None]
        a = qtiles[kp // 4]
        tq = a * 128 + np.arange(128)[None, :]
        masks[kp] = (tk <= tq).astype(BF16)
    return masks


def pack_dr(W):
    """[K, N] fp32 -> DoubleRow-paired fp8 [K/256, 128, 2, N], pre-scaled.
    Logical k = 256*kt + 128*r + p."""
    K, N = W.shape
    Wp = (np.asarray(W, np.float32) * WS).reshape(K // 256, 2, 128, N)
    return np.ascontiguousarray(Wp.transpose(0, 2, 1, 3)).astype(NP_F8)


# ---------------------------------------------------------------------------
# The Bass program (identical for all 8 cores)
# ---------------------------------------------------------------------------
def build_program():
    nc = bacc.Bacc("TRN2")

    d_x = nc.dram_tensor("x_perm", [T, C], F32, kind="ExternalInput")
    d_xob = nc.dram_tensor("x_own_b", [NQ, C], F32, kind="ExternalInput")
    d_masks = nc.dram_tensor("masks", [TT, 128, 128], BF, kind="ExternalInput")
    d_wq = nc.dram_tensor("wq", [KT, 128, 2, C], F8, kind="ExternalInput")
    d_wk = nc.dram_tensor("wk", [KT, 128, 2, C], F8, kind="ExternalInput")
    d_wv = nc.dram_tensor("wv", [KT, 128, 2, C], F8, kind="ExternalInput")
    d_wp = nc.dram_tensor("wp", [C, C], BF, kind="ExternalInput")
    d_wfc = nc.dram_tensor("wfc", [C, DFF], BF, kind="ExternalInput")
    d_wfc2 = nc.dram_tensor("wfc2", [DFF, C], BF, kind="ExternalInput")
    # small per-partition constants packed into one tensor:
    # [l1g l1b l2g l2b bq bk] each [128, CT]
    d_cn = nc.dram_tensor("consts6", [128, 6 * CT], F32, kind="ExternalInput")
    # [bv_bc bfc2_bc] packed: [128, 2*C]
    d_cb = nc.dram_tensor("constsb", [128, 2 * C], F32, kind="ExternalInput")
    d_bfc = nc.dram_tensor("bfc2d", [128, FT], F32, kind="ExternalInput")
    d_ident = nc.dram_tensor("ident", [128, 128], BF, kind="ExternalInput")
    d_out = nc.dram_tensor("out", [NQ, C], F32, kind="ExternalOutput")

    with tile.TileContext(nc) as tc:
        _body(nc, tc, locals())
    nc.compile()
    return nc


def _ln_tile(nc, pool, x_ap, eps):
    """LayerNorm stats for one [128, C] fp32 tile -> (mu, rstd) [128,1]."""
    stats = pool.tile([128, 3, 6], F32, tag="bnstats", name="bnstats")
    xg = x_ap.rearrange("p (a b) -> p a b", b=256)
    for a in range(3):
        nc.vector.bn_stats(out=stats[:, a, :], in_=xg[:, a, :])
    mv = pool.tile([128, 2], F32, tag="bnaggr", name="bnaggr")
    nc.vector.bn_aggr(out=mv[:], in_=stats[:])
    sd = pool.tile([128, 1], F32, tag="sd", name="sd")
    nc.scalar.activation(out=sd[:], in_=mv[:, 1:2], func=AF.Sqrt, bias=eps[:])
    rstd = pool.tile([128, 1], F32, tag="rstd", name="rstd")
    nc.vector.reciprocal(out=rstd[:], in_=sd[:])
    return mv[:, 0:1], rstd


def _body(nc, tc, d):
    def pool(name, **kw):
        return tc.tile_pool(name=name, **kw)

    with (
        pool("const", bufs=1) as constp,
        pool("persist", bufs=1) as pers,
        pool("small", bufs=6) as small,
    ):
        # ---- constants (batched DMAs) -----------------------------------
        ident = constp.tile([128, 128], BF)
        nc.sync.dma_start(ident[:], d["d_ident"][:])
        eps = constp.tile([128, 1], F32)
        nc.vector.memset(eps[:], 1e-5)
        cn = constp.tile([128, 6, CT], F32)
        nc.sync.dma_start(cn[:], d["d_cn"][:].rearrange("p (a b) -> p a b", b=CT))
        l1g, l1b, l2g, l2b, bq, bk = (cn[:, i, :] for i in range(6))
        cb = constp.tile([128, 2, C], F32)
        nc.sync.dma_start(cb[:], d["d_cb"][:].rearrange("p (a b) -> p a b", b=C))
        bv_bc, bfc2_bc = cb[:, 0, :], cb[:, 1, :]

        # ---- persistent activations -----------------------------------
        wp_t = pers.tile([128, CT, C], BF, tag="wp", name="wp_t")
        wp = [wp_t[:, t, :] for t in range(CT)]
        wfc_t = pers.tile([128, CT, DFF], BF, tag="wfc", name="wfc_t")
        wfc = [wfc_t[:, t, :] for t in range(CT)]
        xo_t = pers.tile([128, 4, C], F32, tag="xo", name="xo_t")
        x_own = [xo_t[:, j, :] for j in range(4)]
        qT = [pers.tile([128, NQ], BF, tag=f"qT{t}", name=f"qT{t}") for t in range(CT)]
        yT = [pers.tile([128, NQ], BF, tag=f"yT{t}", name=f"yT{t}") for t in range(CT)]

        with pool("attn_sb", bufs=1) as attnp:
            kT = [attnp.tile([128, T], BF, tag=f"kT{t}", name=f"kT{t}") for t in range(CT)]
            V = [attnp.tile([128, H * (HD + 32), ], BF, tag=f"V{t}", name=f"V{t}") for t in range(TT)]
            masks_t = attnp.tile([128, TT, 128], BF, tag="masks", name="masks_t")
            masks = [masks_t[:, t, :] for t in range(TT)]

            # ======== phase 1: LN1 + transpose to feature-major fp8 ========
            with (
                pool("ph1", bufs=1) as ph1p,
                pool("ph1s", bufs=4) as ph1s,
                pool("wqkv", bufs=1) as wp_,
            ):
                xnT8 = [ph1p.tile([128, 2, T], F8, tag=f"xnT{t}", name=f"xnT{t}") for t in range(KT)]
                xnTq8 = [ph1p.tile([128, 2, NQ], F8, tag=f"xnTq{t}", name=f"xnTq{t}") for t in range(KT)]
                wq8 = [wp_.tile([128, 2, C], F8, tag=f"wq{t}", name=f"wq{t}") for t in range(KT)]
                wk8 = [wp_.tile([128, 2, C], F8, tag=f"wk{t}", name=f"wk{t}") for t in range(KT)]
                wv8 = [wp_.tile([128, 2, C], F8, tag=f"wv{t}", name=f"wv{t}") for t in range(KT)]
                for t in range(KT):
                    nc.sync.dma_start(wk8[t][:], d["d_wk"][t, :, :, :])
                for t in range(KT):
                    nc.sync.dma_start(wv8[t][:], d["d_wv"][t, :, :, :])
                for t in range(KT):
                    nc.sync.dma_start(wq8[t][:], d["d_wq"][t, :, :, :])
                nc.sync.dma_start(
                    masks_t[:], d["d_masks"][:].rearrange("k p c -> p k c")
                )
                nc.sync.dma_start(
                    wp_t[:], d["d_wp"][:].rearrange("(t p) c -> p t c", p=128)
                )
                nc.sync.dma_start(
                    wfc_t[:], d["d_wfc"][:].rearrange("(t p) c -> p t c", p=128)
                )
                nc.gpsimd.dma_start(
                    xo_t[:], d["d_xob"][:].rearrange("(t p) c -> p t c", p=128)
                )
                with (
                    pool("ph1t", bufs=1, space="PSUM") as ph1t,
                    pool("ph2k", bufs=2, space="PSUM") as ph2k,
                ):
                    # PE warm-up: keep the HAM activity monitor at full clock
                    # while the first LayerNorm group is still in flight
                    warm = ph2k.tile([128, 512], F32, tag="pqk", name="warm")
                    for _ in range(128):
                        nc.tensor.matmul(warm[:, 0:128], ident[:], ident[:])
                    for ttg in range(4):
                        ptb = [
                            ph1t.tile([128, 512], BF, tag=f"ptb{t}", name=f"ptb{t}")
                            for t in range(CT)
                        ]
                        for ti in range(4):
                            tt = ttg * 4 + ti
                            xt_t = ph1s.tile([128, C], F32, tag="xt", name="xt")
                            nc.scalar.dma_start(
                                xt_t[:], d["d_x"][tt * 128 : (tt + 1) * 128, :]
                            )
                            xt = xt_t[:]
                            mu, rstd = _ln_tile(nc, small, xt, eps)
                            xn = ph1s.tile([128, C], BF, tag="xn", name="xn")
                            nc.vector.tensor_scalar(
                                out=xn[:], in0=xt, scalar1=mu, scalar2=rstd[:],
                                op0=ALU.subtract, op1=ALU.mult,
                            )
                            for ct in range(CT):
                                nc.tensor.transpose(
                                    ptb[ct][:, ti * 128 : (ti + 1) * 128],
                                    xn[:, ct * 128 : (ct + 1) * 128], ident[:],
                                )
                        for ct in range(CT):
                            kt, r = ct // 2, ct % 2
                            nc.scalar.activation(
                                out=xnT8[kt][:, r, ttg * 512 : (ttg + 1) * 512],
                                in_=ptb[ct][:], func=AF.Identity,
                                scale=l1g[:, ct : ct + 1], bias=l1b[:, ct : ct + 1],
                            )
                            # own q-tile of this group sits at ti == 3
                            nc.scalar.activation(
                                out=xnTq8[kt][:, r, ttg * 128 : (ttg + 1) * 128],
                                in_=ptb[ct][:, 384:512], func=AF.Identity,
                                scale=l1g[:, ct : ct + 1], bias=l1b[:, ct : ct + 1],
                            )
                        # kT chunk ttg depends only on this token-group
                        for f in range(CT):
                            ps = ph2k.tile([128, 512], F32, tag="pqk", name="pk")
                            for kt in range(KT):
                                nc.tensor.matmul(
                                    ps[:], wk8[kt][:, :, f * 128 : (f + 1) * 128],
                                    xnT8[kt][:, :, ttg * 512 : (ttg + 1) * 512],
                                    start=(kt == 0), stop=(kt == KT - 1),
                                    perf_mode=DR,
                                )
                            nc.vector.tensor_scalar(
                                out=kT[f][:, ttg * 512 : (ttg + 1) * 512],
                                in0=ps[:], scalar1=bk[:, f : f + 1],
                                scalar2=None, op0=ALU.add,
                            )

                # ======== phase 2: Q^T, V (fp8 DoubleRow) ========
                with (
                    pool("ph2ps", bufs=3, space="PSUM") as ph2ps,
                    pool("ph2pv", bufs=2, space="PSUM") as ph2pv,
                ):
                    for nn in range(4):
                        for tt in range(nn * 4, nn * 4 + 4):
                            pv = ph2pv.tile([128, C], F32, tag="pv", name="pv")
                            for lo, hi in ((0, 512), (512, 768)):
                                for kt in range(KT):
                                    nc.tensor.matmul(
                                        pv[:, lo:hi],
                                        xnT8[kt][:, :, tt * 128 : (tt + 1) * 128],
                                        wv8[kt][:, :, lo:hi],
                                        start=(kt == 0), stop=(kt == KT - 1),
                                        perf_mode=DR,
                                    )
                            vt = V[tt][:].rearrange("p (h e) -> p h e", e=HD + 32)
                            # 32 replicated "ones" columns per head: the AV
                            # matmul then lands the softmax denominator in
                            # psum rows 64:96 (value WS cancels the fp8
                            # weight scale between numerator and denominator)
                            nc.vector.memset(vt[:, :, HD : HD + 32], WS)
                            pvh = pv[:].rearrange("p (h e) -> p h e", e=HD)
                            nc.vector.tensor_tensor(
                                out=vt[:, :, 0:HD], in0=pvh[:],
                                in1=bv_bc[:].rearrange("p (h e) -> p h e", e=HD),
                                op=ALU.add,
                            )
                    # qT[f] [128, 512] = (Wq[:, f].T @ xnTq) + bq
                    for f in range(CT):
                        ps = ph2ps.tile([128, NQ], F32, tag="pqk", name="pq")
                        for kt in range(KT):
                            nc.tensor.matmul(
                                ps[:], wq8[kt][:, :, f * 128 : (f + 1) * 128],
                                xnTq8[kt][:], start=(kt == 0), stop=(kt == KT - 1),
                                perf_mode=DR,
                            )
                        nc.vector.tensor_scalar(
                            out=qT[f][:], in0=ps[:], scalar1=bq[:, f : f + 1],
                            scalar2=None, op0=ALU.add,
                        )

            # ======== phase 3: attention (bf16, 4 heads in flight) ========
            # exp is batched over head PAIRS (one ACT op per pair) since ACT
            # runs 1x with a 352-cycle fixed cost per instruction.
            with (
                pool("ph3", bufs=8) as ph3s,
                pool("ph3ps", bufs=2, space="PSUM") as ph3ps,
                pool("ph3pa", bufs=1, space="PSUM") as ph3pa,
            ):
                for hg in range(H // 4):
                    hs = [hg * 4 + i for i in range(4)]
                    pavs = {
                        h: ph3pa.tile(
                            [128, NQ], F32, tag=f"pav{h % 4}", name=f"pav{h % 4}"
                        )
                        for h in hs
                    }
                    for kp in range(TT):
                        cs = 128 * (kp // 4)
                        psbs = {}
                        for pi in range(2):
                            hA, hB = hs[2 * pi], hs[2 * pi + 1]
                            ps2 = ph3ps.tile([128, 2, NQ], F32, tag="ps2", name="ps2")
                            for r, h in ((0, hA), (1, hB)):
                                ro = (h % 2) * 64
                                nc.tensor.matmul(
                                    ps2[:, r, cs:NQ],
                                    kT[h // 2][ro : ro + 64, kp * 128 : (kp + 1) * 128],
                                    qT[h // 2][ro : ro + 64, cs:NQ],
                                )
                            p_sb = ph3s.tile([128, 2, NQ], BF, tag="p_sb", name="p_sb")
                            # q,k both carry the 32x fp8 weight scale
                            nc.scalar.activation(
                                out=p_sb[:, :, cs:NQ], in_=ps2[:, :, cs:NQ],
                                func=AF.Exp, scale=0.125 / (WS * WS),
                            )
                            # only the first in-suffix 128-col block is ever
                            # not all-ones (across every core layout)
                            for r, h in ((0, hA), (1, hB)):
                                nc.vector.tensor_mul(
                                    p_sb[:, r, cs : cs + 128],
                                    p_sb[:, r, cs : cs + 128], masks[kp][:],
                                )
                                psbs[h] = (p_sb, r)
                        for h in hs:
                            p_sb, r = psbs[h]
                            nc.tensor.matmul(
                                pavs[h][0 : HD + 32, cs:NQ],
                                V[kp][:, h * (HD + 32) : (h + 1) * (HD + 32)],
                                p_sb[:, r, cs:NQ],
                                start=(kp == 0), stop=(kp == TT - 1),
                                skip_group_check=True,
                            )
                    # gather the 4 heads' denominators and invert them in a
                    # single wide reciprocal (DVE reciprocal cost is per
                    # free-element, independent of partition count)
                    denall = ph3s.tile([128, NQ], F32, tag="denall", name="denall")
                    for i, h in enumerate(hs):
                        nc.vector.tensor_copy(
                            denall[32 * i : 32 * (i + 1), :],
                            pavs[h][HD : HD + 32, :],
                        )
                    rball = ph3s.tile([128, NQ], F32, tag="rball", name="rball")
                    nc.vector.reciprocal(out=rball[:], in_=denall[:])
                    for i, h in enumerate(hs):
                        ro = (h % 2) * 64
                        for half in range(2):
                            nc.vector.tensor_tensor(
                                out=yT[h // 2][ro + 32 * half : ro + 32 * (half + 1), :],
                                in0=pavs[h][32 * half : 32 * (half + 1), :],
                                in1=rball[32 * i : 32 * (i + 1), :], op=ALU.mult,
                            )

        # ======== phase 4: proj + residual + LN2 ========
        with pool("mlp_sb", bufs=1) as mlpp:
            wfc2A = mlpp.tile([128, FT // 2, C], BF, tag="wfc2A", name="wfc2A")
            nc.sync.dma_start(
                wfc2A[:],
                d["d_wfc2"][: FT // 2 * 128, :].rearrange("(t p) c -> p t c", p=128),
            )
            x2 = [mlpp.tile([128, C], F32, tag=f"x2{j}", name=f"x2{j}") for j in range(4)]
            xn2T = [mlpp.tile([128, NQ], BF, tag=f"xn2T{t}", name=f"xn2T{t}") for t in range(CT)]
            hT = [mlpp.tile([128, NQ], BF, tag=f"hT{t}", name=f"hT{t}") for t in range(FT)]
            with (
                pool("mlp1", bufs=1) as m1p,
                pool("mlp1s", bufs=3) as m1s,
            ):
                bfcc = m1p.tile([128, FT], F32, tag="bfcc", name="bfcc")
                nc.sync.dma_start(bfcc[:], d["d_bfc"][:])

                with (
                    pool("ph4p", bufs=2, space="PSUM") as ph4p,
                    pool("ph4t", bufs=4, space="PSUM") as ph4t,
                ):
                    for qt in range(4):
                        pp = ph4p.tile([128, C], F32, tag="pp", name="pp")
                        for lo, hi in ((0, 512), (512, 768)):
                            for ct in range(CT):
                                nc.tensor.matmul(
                                    pp[:, lo:hi],
                                    yT[ct][:, qt * 128 : (qt + 1) * 128],
                                    wp[ct][:, lo:hi],
                                    start=(ct == 0), stop=(ct == CT - 1),
                                )
                        nc.vector.tensor_add(x2[qt][:], pp[:], x_own[qt][:])
                        mu, rstd = _ln_tile(nc, small, x2[qt][:], eps)
                        xn2 = m1s.tile([128, C], BF, tag="xn2", name="xn2")
                        nc.vector.tensor_scalar(
                            out=xn2[:], in0=x2[qt][:], scalar1=mu, scalar2=rstd[:],
                            op0=ALU.subtract, op1=ALU.mult,
                        )
                        for ct in range(CT):
                            pt = ph4t.tile([128, 128], BF, tag="pt4", name="pt4")
                            nc.tensor.transpose(
                                pt[:], xn2[:, ct * 128 : (ct + 1) * 128], ident[:]
                            )
                            nc.scalar.activation(
                                out=xn2T[ct][:, qt * 128 : (qt + 1) * 128],
                                in_=pt[:], func=AF.Identity,
                                scale=l2g[:, ct : ct + 1], bias=l2b[:, ct : ct + 1],
                            )

                # ======== phase 5: fc -> hT directly (feature-major out),
                # gelu bias is then per-partition ========
                with pool("ph5p", bufs=3, space="PSUM") as ph5p:
                    for f in range(FT):
                        ph_ = ph5p.tile([128, 512], F32, tag="ph5", name="ph5")
                        for ct in range(CT):
                            nc.tensor.matmul(
                                ph_[:],
                                wfc[ct][:, f * 128 : (f + 1) * 128],
                                xn2T[ct][:],
                                start=(ct == 0), stop=(ct == CT - 1),
                            )
                        nc.scalar.activation(
                            out=hT[f][:], in_=ph_[:], func=AF.Gelu_apprx_tanh,
                            bias=bfcc[:, f : f + 1],
                        )

            # ======== phase 7: fc2 + residual + out ========
            with (
                pool("mlp2", bufs=1) as m2p,
                pool("mlp2s", bufs=3) as m2s,
                pool("ph7p", bufs=2, space="PSUM") as ph7p,
            ):
                wfc2B = m2p.tile([128, FT // 2, C], BF, tag="wfc2B", name="wfc2B")
                nc.sync.dma_start(
                    wfc2B[:],
                    d["d_wfc2"][FT // 2 * 128 :, :].rearrange(
                        "(t p) c -> p t c", p=128
                    ),
                )
                wfc2 = [wfc2A[:, t, :] for t in range(FT // 2)] + [
                    wfc2B[:, t, :] for t in range(FT // 2)
                ]
                for qt in range(4):
                    po = ph7p.tile([128, C], F32, tag="po", name="po")
                    for lo, hi in ((0, 512), (512, 768)):
                        for kt in range(FT):
                            nc.tensor.matmul(
                                po[:, lo:hi],
                                hT[kt][:, qt * 128 : (qt + 1) * 128],
                                wfc2[kt][:, lo:hi],
                                start=(kt == 0), stop=(kt == FT - 1),
                            )
                    t1 = m2s.tile([128, C], F32, tag="t1", name="t1")
                    nc.vector.tensor_add(t1[:], po[:], bfc2_bc[:])
                    ot = m2s.tile([128, C], F32, tag="ot", name="ot")
                    nc.vector.tensor_add(ot[:], t1[:], x2[qt][:])
                    nc.sync.dma_start(
                        d["d_out"][qt * 128 : (qt + 1) * 128, :], ot[:]
                    )


# ---------------------------------------------------------------------------
# Host-side wrapper
# ---------------------------------------------------------------------------
_PROGRAM = None


def _get_program():
    global _PROGRAM
    if _PROGRAM is None:
        _PROGRAM = build_program()
    return _PROGRAM


def make_in_maps(x, ln1_g, ln1_b, W_attn, b_attn, W_proj, b_proj,
                 ln2_g, ln2_b, W_fc, b_fc, W_fc2, b_fc2):
    x = np.asarray(x, np.float32)
    shared = {
        "wq": pack_dr(W_attn[:, 0:C]),
        "wk": pack_dr(W_attn[:, C : 2 * C]),
        "wv": pack_dr(W_attn[:, 2 * C : 3 * C]),
        "wp": np.asarray(W_proj, BF16),
        "wfc": np.asarray(W_fc, BF16),
        "wfc2": np.asarray(W_fc2, BF16),
        # consts6 = [l1g l1b l2g l2b bq bk]; q/k/v biases ride the 32x scale
        "consts6": np.concatenate([
            np.asarray(v, np.float32).reshape(CT, 128).T
            for v in (ln1_g, ln1_b, ln2_g, ln2_b,
                      np.asarray(b_attn[0:C], np.float32) * WS,
                      np.asarray(b_attn[C : 2 * C], np.float32) * WS)
        ], axis=1).copy(),
        "constsb": np.concatenate([
            np.broadcast_to(
                np.asarray(b_attn[2 * C : 3 * C], np.float32) * WS, (128, C)),
            np.broadcast_to(np.asarray(b_fc2, np.float32), (128, C)),
        ], axis=1).copy(),
        "bfc2d": np.ascontiguousarray(
            np.asarray(b_fc, np.float32).reshape(FT, 128).T),
        "ident": np.eye(128, dtype=BF16),
    }
    bp = np.asarray(b_proj, np.float32)
    in_maps, layouts = [], []
    for core in range(8):
        b, g = core // 4, core % 4
        qtiles, perm = core_layout(g)
        idx = np.concatenate([np.arange(t * 128, (t + 1) * 128) for t in perm])
        own = np.concatenate([np.arange(t * 128, (t + 1) * 128) for t in qtiles])
        m = dict(shared)
        m["x_perm"] = np.ascontiguousarray(x[b][idx])
        m["x_own_b"] = np.ascontiguousarray(x[b][own] + bp)
        m["masks"] = core_masks(qtiles, perm)
        in_maps.append(m)
        layouts.append((b, own))
    return in_maps, layouts


def unshard(results, layouts):
    out = np.empty((B, T, C), np.float32)
    for r, (b, own) in zip(results, layouts):
        out[b][own] = r["out"]
    return out


def kernel(**inputs):
    from concourse.bass_utils import run_bass_kernel_spmd

    nc = _get_program()
    in_maps, layouts = make_in_maps(**inputs)
    res = run_bass_kernel_spmd(nc, in_maps, core_ids=list(range(8)))
    return unshard(res.results, layouts)
